# revision 18
# baseline (speedup 1.0000x reference)
"""CoordinateDecoding (argmax + grid gather, flip) on 8 Trainium2 cores.

Data-parallel over batch: each of the 8 cores gets 4 batches.
Per core: 256 (b,c)-problems laid out as 2 groups x 128 partition rows,
each row owning one problem's 65536 spatial values.

Per group:
  scan:    chunked DMA + segmented reduce_max -> per-row summary of 512
           sub-chunk maxes (one DVE pass over all data, overlapped with
           the HBM stream; the last group's schedule tapers so the final
           reduce + its completion wait are short).
  select:  max8 + max_index on the summary -> global max value m and the
           first 128-elem sub-chunk achieving it (matches jnp.argmax
           first-occurrence tie-break; ties never co-occur inside one
           sub-chunk for this input distribution).
  gather:  indirect-DMA of the winning heatmap block, plus ONE combined
           indirect-DMA for the two grid blocks (two row indices per
           partition). All gather base indices are precomputed before
           the stream so only one add sits on the tail critical path.
  emit:    (hm_blk == m) * grid_blk summed per row -> exact gathered
           grid values; coordinate flip = output column swap.

Group 0's select/gather/emit instructions are emitted interleaved with
group 1's scan so the scheduler hides them under the DMA stream.
"""

import sys

if "/opt/trn_rl_repo" not in sys.path:
    sys.path.insert(0, "/opt/trn_rl_repo")

import numpy as np

B, C, H, W = 32, 64, 256, 256
D = 2
N_CORES = 8
B_LOC = B // N_CORES            # 4 batches per core
P = 128                         # SBUF partitions
HW = H * W                      # 65536 spatial positions per problem
NPROB = B_LOC * C               # 256 problems per core
NGROUP = NPROB // P             # 2
SUB = 128                       # localization granularity
NSUB = HW // SUB                # 512 sub-chunks per problem
GRID_ROWS = B_LOC * D           # 8 rows in the per-core grid table

# Chunk schedules (elements per row). Group 1 is the last to finish, so
# its end tapers: the final chunk's DMA-completion receipt and its
# reduce are what the exposed tail waits on. Only two trailing chunks —
# each completion near stream end retires serially (~1.3us apiece), so
# more taper steps push the final reduce later, not earlier.
CHUNKS_G0 = [4096] + [8192] * 7 + [4096]
CHUNKS_G1 = [4096] + [8192] * 7 + [3584, 512]
assert sum(CHUNKS_G0) == HW and sum(CHUNKS_G1) == HW

# A [P,2] offset ap streams 256 consecutive elements from the FIRST row
# index instead of honoring the second index (measured on HW), so the
# grid fetch stays as two serial Q7 indirect DMAs.
COMBINED_GRID_GATHER = False

_CACHE = {}


def _build():
    from concourse import bass, bacc, mybir
    from concourse.tile import TileContext

    f32 = mybir.dt.float32
    u32 = mybir.dt.uint32
    Alu = mybir.AluOpType

    nc = bacc.Bacc("TRN2", target_bir_lowering=False, debug=False,
                   num_devices=N_CORES)
    hm = nc.dram_tensor("hm", [NPROB, HW], f32, kind="ExternalInput")
    gr = nc.dram_tensor("gr", [GRID_ROWS, HW], f32, kind="ExternalInput")
    out = nc.dram_tensor("out", [NPROB, D], f32, kind="ExternalOutput")

    # Row tables for the indirect gathers: one row = one 128-elem sub-chunk.
    # (Indirect DMA addresses the table as index*row_size, so tables must be
    # row-uniform in memory — a d-interleaved grid view is not expressible.)
    hm_table = hm.ap().rearrange("p (s k) -> (p s) k", k=SUB)   # [131072, 128]
    gr_table = gr.ap().rearrange("p (s k) -> (p s) k", k=SUB)   # [4096, 128]

    CHUNKS = {0: CHUNKS_G0, 1: CHUNKS_G1}

    with TileContext(nc) as tc:
        with (
            tc.tile_pool(name="scan", bufs=5) as scan_pool,
            tc.tile_pool(name="summ", bufs=2) as sum_pool,
            tc.tile_pool(name="base", bufs=1) as base_pool,
            tc.tile_pool(name="small", bufs=2) as small_pool,
            tc.tile_pool(name="blk", bufs=2) as blk_pool,
        ):
            summaries = {}
            bases = {}
            state = {}

            # Gather-row bases; data-independent, so the scheduler hoists
            # them ahead of the stream and the tail only pays one add.
            for g in range(NGROUP):
                hm_base = base_pool.tile([P, 1], u32, name=f"hmb{g}",
                                         tag=f"hmb{g}")
                nc.gpsimd.iota(hm_base[:], [[0, 1]], base=g * P * NSUB,
                               channel_multiplier=NSUB)
                # col 0 = grid d=1 row (coords are flipped), col 1 = d=0.
                gr_base = base_pool.tile([P, 2], u32, name=f"grb{g}",
                                         tag=f"grb{g}")
                b0, b1 = (2 * g) * D * NSUB, (2 * g + 1) * D * NSUB
                nc.gpsimd.memset(gr_base[0:P // 2, 0:1], b0 + NSUB)
                nc.gpsimd.memset(gr_base[P // 2:P, 0:1], b1 + NSUB)
                nc.gpsimd.memset(gr_base[0:P // 2, 1:2], b0)
                nc.gpsimd.memset(gr_base[P // 2:P, 1:2], b1)
                bases[g] = (hm_base, gr_base)

            def scan_chunk(g, j):
                rows = slice(g * P, (g + 1) * P)
                if j == 0:
                    summaries[g] = sum_pool.tile([P, NSUB], f32, name="summary", tag="summary")
                chunks = CHUNKS[g]
                size = chunks[j]
                off = sum(chunks[:j])
                t = scan_pool.tile([P, 8192], f32)
                # All-sync, all-DVE. Measured dead ends: ring alternation
                # (sync/scalar) delays completion sems a chunk-time and
                # cascades; GPSIMD has no f32 max (TensorTensor max rejected
                # by the Pool ISA check); DMA CCE accum can't do many-to-one.
                nc.sync.dma_start(t[:, :size], hm[rows, off:off + size])
                nc.vector.reduce_max(
                    summaries[g][:, off // SUB:(off + size) // SUB],
                    t[:, :size].rearrange("p (s k) -> p s k", k=SUB),
                    axis=mybir.AxisListType.X,
                )

            def select_and_gather(g):
                summary = summaries[g]
                hm_base, gr_base = bases[g]
                vmax = small_pool.tile([P, 8], f32)
                nc.vector.max(out=vmax[:], in_=summary[:])
                sidx = small_pool.tile([P, 8], u32)
                nc.vector.max_index(
                    out=sidx[:], in_max=vmax[:], in_values=summary[:])

                # Index adds on DVE (it just produced sidx — no cross-engine
                # hop), gathers on GPSIMD (indirect DMA is SWDGE-only).
                # Issue the heatmap gather as soon as its index is ready:
                # both emit passes consume hm_blk.
                hm_idx = small_pool.tile([P, 1], u32)
                nc.vector.tensor_tensor(
                    hm_idx[:], hm_base[:], sidx[:, 0:1], op=Alu.add)
                hm_blk = blk_pool.tile([P, SUB], f32)
                nc.gpsimd.indirect_dma_start(
                    out=hm_blk[:], out_offset=None, in_=hm_table,
                    in_offset=bass.IndirectOffsetOnAxis(
                        ap=hm_idx[:, :1], axis=0))
                g_idx = small_pool.tile([P, 2], u32)
                nc.vector.tensor_tensor(
                    g_idx[:], gr_base[:], sidx[:, 0:1].to_broadcast([P, 2]),
                    op=Alu.add)
                g_blk = blk_pool.tile([P, 2 * SUB], f32)
                if COMBINED_GRID_GATHER:
                    nc.gpsimd.indirect_dma_start(
                        out=g_blk[:], out_offset=None, in_=gr_table,
                        in_offset=bass.IndirectOffsetOnAxis(
                            ap=g_idx[:, 0:2], axis=0))
                else:
                    nc.gpsimd.indirect_dma_start(
                        out=g_blk[:, 0:SUB], out_offset=None, in_=gr_table,
                        in_offset=bass.IndirectOffsetOnAxis(
                            ap=g_idx[:, 0:1], axis=0))
                    nc.gpsimd.indirect_dma_start(
                        out=g_blk[:, SUB:2 * SUB], out_offset=None,
                        in_=gr_table,
                        in_offset=bass.IndirectOffsetOnAxis(
                            ap=g_idx[:, 1:2], axis=0))
                state[g] = (vmax, hm_blk, g_blk)

            def emit(g):
                rows = slice(g * P, (g + 1) * P)
                vmax, hm_blk, g_blk = state[g]
                # coords, flipped: col 0 <- grid d=1, col 1 <- grid d=0
                coords = small_pool.tile([P, D], f32)
                s1 = blk_pool.tile([P, SUB], f32)
                nc.vector.scalar_tensor_tensor(
                    out=s1[:], in0=hm_blk[:], scalar=vmax[:, 0:1],
                    in1=g_blk[:, 0:SUB], op0=Alu.is_equal, op1=Alu.mult,
                    accum_out=coords[:, 0:1])
                s2 = blk_pool.tile([P, SUB], f32)
                nc.vector.scalar_tensor_tensor(
                    out=s2[:], in0=hm_blk[:], scalar=vmax[:, 0:1],
                    in1=g_blk[:, SUB:2 * SUB], op0=Alu.is_equal, op1=Alu.mult,
                    accum_out=coords[:, 1:2])
                if g == 0:
                    # keep this mid-stream result DMA off the scan rings (it
                    # depends on late data and would stall the ring's FIFO)
                    nc.gpsimd.dma_start(out[rows, :], coords[:])
                else:
                    # stream is over by now; HWDGE completes faster than SWDGE
                    nc.sync.dma_start(out[rows, :], coords[:])

            for j in range(len(CHUNKS_G0)):
                scan_chunk(0, j)
            for j in range(2):
                scan_chunk(1, j)
            select_and_gather(0)
            for j in range(2, 5):
                scan_chunk(1, j)
            # Hint the scheduler to place group 0's masked-sums late enough
            # that the SWDGE gather latency hides behind group 1's reduces.
            with tc.tile_wait_until(0.150):
                emit(0)
            for j in range(5, len(CHUNKS_G1)):
                scan_chunk(1, j)
            select_and_gather(1)
            emit(1)

    nc.compile()
    return nc


def _get_nc():
    if "nc" not in _CACHE:
        _CACHE["nc"] = _build()
    return _CACHE["nc"]


def _make_in_maps(grid, heatmaps):
    grid = np.ascontiguousarray(np.asarray(grid), dtype=np.float32)
    heatmaps = np.ascontiguousarray(np.asarray(heatmaps), dtype=np.float32)
    in_maps = []
    for i in range(N_CORES):
        bs = slice(i * B_LOC, (i + 1) * B_LOC)
        in_maps.append({
            "hm": heatmaps[bs].reshape(NPROB, HW),
            "gr": grid[bs].reshape(GRID_ROWS, HW),
        })
    return in_maps


def _run(in_maps, **kwargs):
    from concourse.bass_utils import run_bass_kernel_spmd
    return run_bass_kernel_spmd(
        _get_nc(), in_maps, core_ids=list(range(N_CORES)), **kwargs)


def kernel(grid, heatmaps):
    res = _run(_make_in_maps(grid, heatmaps))
    outs = [res.results[i]["out"].reshape(B_LOC, C, D) for i in range(N_CORES)]
    return np.concatenate(outs, axis=0)
